# revision 1
# baseline (speedup 1.0000x reference)
"""Multi-head attention (B=2, S=2048, D=1024, H=16, dk=64) on 8 TRN2 cores.

Sharding: core c -> (batch b = c//4, head-group g = c%4 of 4 heads).
Each core computes q/k/v projections for its 4 heads, full attention for
those heads, and a partial output projection (rows g*256:(g+1)*256 of Wo).
Host pre-transposes/casts inputs to bf16 and sums the partial outputs.

Device layout (per core, all matmul operands bf16, accumulation f32):
  xqT/xkT/xvT [1024, 2048]   (d on partitions -> contraction-ready)
  qT, kT      [256, 2048]    (head-dim on partitions; pair tiles [128, S])
  v_aug       [2048, 4*65]   (per head: [v_h | ones]; ones col => softmax denom)
  scoresT     [j, i] in PSUM; exp on ScalarE -> probsT bf16 (no max-subtract:
              scores ~ N(0,1) after 1/8 scaling, exp bounded ~e^6)
  PV:         attnT_unnorm[e, i] = sum_j v_aug[j, e] * probsT[j, i]
              (row 64 = softmax denominator), normalize via reciprocal +
              K=1 broadcast matmul, store attnT [64, S] per head
  out-projT:  outT[n, s] = sum_{h,e} wo[h,e,n] * attnT_h[e, s]  (K=64 x4)
Host: out[b] = sum_g outT_partial.T + (bv @ Wo + bo).
"""

import os

import numpy as np
import ml_dtypes

BF16 = ml_dtypes.bfloat16

B, S, D = 2, 2048, 1024
H, DK = 16, 64
P = 128
GROUPS = 4          # head groups (one per core within a batch)
HPG = 4             # heads per group
GD = HPG * DK       # 256, group width
KC = D // P         # 8 contraction chunks
ST = S // P         # 16 s-tiles / j-tiles
NCORES = 8
FP8_PV = False      # fp8 PV measured 3.7e-2 rel err (e4m3 noise) - keep bf16
DEBUG_DUMP = False  # extra outputs: per-head attnT and denominators

_cached = {}


def _build_bass():
    import concourse.bass as bass
    import concourse.tile as tile
    from concourse.bacc import Bacc
    from concourse import mybir
    from contextlib import ExitStack

    f32 = mybir.dt.float32
    bf16 = mybir.dt.bfloat16
    Act = mybir.ActivationFunctionType

    nc = Bacc()

    xqT = nc.dram_tensor("xqT", [D, S], bf16, kind="ExternalInput")
    xkT = nc.dram_tensor("xkT", [D, S], bf16, kind="ExternalInput")
    xvT = nc.dram_tensor("xvT", [D, S], bf16, kind="ExternalInput")
    wq = nc.dram_tensor("wq", [D, GD], bf16, kind="ExternalInput")
    wk = nc.dram_tensor("wk", [D, GD], bf16, kind="ExternalInput")
    wv = nc.dram_tensor("wv", [D, GD], bf16, kind="ExternalInput")
    wo = nc.dram_tensor("wo", [GD, D], bf16, kind="ExternalInput")
    bq = nc.dram_tensor("bq", [GD, 1], f32, kind="ExternalInput")
    bk = nc.dram_tensor("bk", [GD, 1], f32, kind="ExternalInput")
    out = nc.dram_tensor("out", [S, D], f32, kind="ExternalOutput")

    with tile.TileContext(nc) as tc, ExitStack() as ctx:
        singles = ctx.enter_context(tc.tile_pool(name="singles", bufs=1))
        probs_pool = ctx.enter_context(tc.tile_pool(name="probs", bufs=3))
        small = ctx.enter_context(tc.tile_pool(name="small", bufs=8))
        outs_pool = ctx.enter_context(tc.tile_pool(name="outs", bufs=8))
        psum = ctx.enter_context(tc.tile_pool(name="psum", bufs=1, space="PSUM"))

        # ---- persistent SBUF ----
        wq_sb = singles.tile([P, KC, GD], bf16)
        wk_sb = singles.tile([P, KC, GD], bf16)
        wv_sb = singles.tile([P, KC, GD], bf16)
        wo_sb = singles.tile([P, 2, D], bf16)
        bq_sb = singles.tile([P, 2, 1], f32)
        bk_sb = singles.tile([P, 2, 1], f32)
        nc.sync.dma_start(out=wq_sb, in_=wq.rearrange("(c p) m -> p c m", p=P))
        nc.sync.dma_start(out=wk_sb, in_=wk.rearrange("(c p) m -> p c m", p=P))
        nc.sync.dma_start(out=wv_sb, in_=wv.rearrange("(c p) m -> p c m", p=P))
        nc.sync.dma_start(out=wo_sb, in_=wo.rearrange("(c p) n -> p c n", p=P))
        nc.sync.dma_start(out=bq_sb, in_=bq.rearrange("(t p) o -> p t o", p=P))
        nc.sync.dma_start(out=bk_sb, in_=bk.rearrange("(t p) o -> p t o", p=P))

        xq_sb = singles.tile([P, KC, S], bf16)
        xk_sb = singles.tile([P, KC, S], bf16)
        xv_sb = singles.tile([P, KC, S], bf16)
        # tensor-by-tensor so q-proj can start after the first xq chunk
        # and PE chases the DMA stream instead of waiting on all three
        for k in range(KC):
            nc.sync.dma_start(out=xq_sb[:, k, :], in_=xqT[k * P:(k + 1) * P, :])
        for k in range(KC):
            nc.sync.dma_start(out=xk_sb[:, k, :], in_=xkT[k * P:(k + 1) * P, :])
        for k in range(KC):
            nc.sync.dma_start(out=xv_sb[:, k, :], in_=xvT[k * P:(k + 1) * P, :])

        qT_sb = [singles.tile([P, S], bf16, name=f"qT{t}") for t in range(2)]
        kT_sb = [singles.tile([P, S], bf16, name=f"kT{t}") for t in range(2)]
        # attnT per head pair [128 hd, S]: even head at partitions 0:64
        # (written directly by DVE), odd head at 64:128 (DVE writes a base-0
        # staging tile, then SBUF->SBUF DMA relocates partitions - engines
        # are lane-locked but DMA is not). Enables K=128 out-projection.
        att_pair = [singles.tile([P, S], bf16, name=f"attp{p}")
                    for p in range(2)]
        att_odd = [singles.tile([DK, S], bf16, name=f"atto{p}")
                   for p in range(2)]

        ones_sb = singles.tile([65, DK], f32)
        nc.vector.memset(ones_sb[64:65, :], 1.0)

        CP = ST // 2
        if FP8_PV:
            fp8 = mybir.dt.float8e4
            # [j-in-chunk, chunk-pair, chunk-in-pair, head, 64 v cols + 1 one + pad]
            v_sb = singles.tile([P, CP, 2, HPG, 80], fp8)
            nc.vector.memset(v_sb[:, :, :, :, 64:65], 1.0)
            v4 = None
            # exp(s/8 - 3): keeps exp within IEEE e4m3 range (max finite 240;
            # max observed score ~7.7 -> e^4.7 ~ 110). Softmax shift-invariant.
            exp_bias = singles.tile([P, 1], f32)
            nc.vector.memset(exp_bias, -3.0)
        else:
            v_sb = singles.tile([P, ST, HPG * 65], bf16)
            # ones columns of v_aug (col 64 of each per-head [64|1] block)
            v4 = v_sb.rearrange("p s (h c) -> p s h c", c=65)
            nc.vector.memset(v4[:, :, :, 64:65], 1.0)

        # ---- phase A: projections ----
        def qk_proj(x_sb, w_sb, b_sb, dst, t):
            pq = [psum.tile([P, 1024], mybir.dt.float32, tag="sc", bufs=2,
                            name=f"pq{t}{half}") for half in range(2)]
            for k in range(KC):
                for half in range(2):
                    for sq in range(2):
                        nc.tensor.matmul(
                            out=pq[half][:, sq * 512:(sq + 1) * 512],
                            lhsT=w_sb[:, k, t * P:(t + 1) * P],
                            rhs=x_sb[:, k, half * 1024 + sq * 512:
                                     half * 1024 + (sq + 1) * 512],
                            start=(k == 0), stop=(k == KC - 1))
            for half in range(2):
                nc.vector.tensor_scalar_add(
                    out=dst[:, half * 1024:(half + 1) * 1024],
                    in0=pq[half], scalar1=b_sb[:, t, :])

        def v_proj():
            for st in range(ST):
                pvv = psum.tile([P, GD], mybir.dt.float32, tag="pv", bufs=4, name="pvv")
                for k in range(KC):
                    nc.tensor.matmul(
                        out=pvv,
                        lhsT=xv_sb[:, k, st * P:(st + 1) * P],
                        rhs=wv_sb[:, k, :],
                        start=(k == 0), stop=(k == KC - 1))
                if FP8_PV:
                    dst = v_sb[:, st // 2, st % 2, :, 0:64]
                else:
                    dst = v4[:, st, :, 0:64]
                src = pvv.rearrange("p (h c) -> p h c", c=64)
                nc.vector.tensor_copy(out=dst, in_=src)

        # ---- phase B: attention for one head pair, one i-half ----
        # `pending` = previous iteration's normalize emitter; it is emitted
        # after this iteration's first two j-tiles so ACT/PE stay fed across
        # the (pair, ih) boundary. Returns this iteration's normalize.
        def attention(pair, ih, pending=None):
            pv = [[psum.tile([65, 512], mybir.dt.float32, tag="pv", bufs=4,
                             name=f"pv{pair}{ih}{hp}{iq}")
                   for iq in range(2)] for hp in range(2)]
            if FP8_PV:
                fp8 = mybir.dt.float8e4
                for cp in range(CP):
                    pr = [probs_pool.tile([P, 2, 1024], fp8, tag="probs",
                                          name=f"pr{hp}") for hp in range(2)]
                    for d in range(2):
                        jt = 2 * cp + d
                        sc = [psum.tile([P, 1024], mybir.dt.float32, tag="sc",
                                        bufs=2, name=f"sc{hp}")
                              for hp in range(2)]
                        for iq in range(2):
                            for hp in range(2):
                                nc.tensor.matmul(
                                    out=sc[hp][:, iq * 512:(iq + 1) * 512],
                                    lhsT=kT_sb[pair][hp * 64:(hp + 1) * 64,
                                                     jt * P:(jt + 1) * P],
                                    rhs=qT_sb[pair][hp * 64:(hp + 1) * 64,
                                                    ih * 1024 + iq * 512:
                                                    ih * 1024 + (iq + 1) * 512],
                                    start=True, stop=True)
                        for hp in range(2):
                            # exp(s/8 - 2): global shift keeps exp within
                            # e4m3 range (softmax is shift-invariant)
                            nc.scalar.activation(out=pr[hp][:, d, :],
                                                 in_=sc[hp], func=Act.Exp,
                                                 scale=0.125, bias=exp_bias)
                    for hp in range(2):
                        h = 2 * pair + hp
                        for iq in range(2):
                            nc.tensor.matmul(
                                out=pv[hp][iq][:, :],
                                lhsT=v_sb[:, cp, :, h, 0:65],
                                rhs=pr[hp][:, :, iq * 512:(iq + 1) * 512],
                                perf_mode=mybir.MatmulPerfMode.DoubleRow,
                                start=(cp == 0), stop=(cp == CP - 1))
                    if cp == 1 and pending is not None:
                        pending()
            else:
                for jt in range(ST):
                    sc = [psum.tile([P, 1024], mybir.dt.float32, tag="sc",
                                    bufs=2, name=f"sc{hp}") for hp in range(2)]
                    for iq in range(2):
                        for hp in range(2):
                            nc.tensor.matmul(
                                out=sc[hp][:, iq * 512:(iq + 1) * 512],
                                lhsT=kT_sb[pair][hp * 64:(hp + 1) * 64,
                                                 jt * P:(jt + 1) * P],
                                rhs=qT_sb[pair][hp * 64:(hp + 1) * 64,
                                                ih * 1024 + iq * 512:
                                                ih * 1024 + (iq + 1) * 512],
                                start=True, stop=True)
                    for hp in range(2):
                        probs = probs_pool.tile([P, 1024], bf16, tag="probs",
                                                name="probs")
                        nc.scalar.activation(out=probs, in_=sc[hp],
                                             func=Act.Exp, scale=0.125)
                        h65 = (2 * pair + hp) * 65
                        for iq in range(2):
                            nc.tensor.matmul(
                                out=pv[hp][iq][:, :],
                                lhsT=v_sb[:, jt, h65:h65 + 65],
                                rhs=probs[:, iq * 512:(iq + 1) * 512],
                                start=(jt == 0), stop=(jt == ST - 1))
                    if jt == 1 and pending is not None:
                        pending()

            def normalize():
                for hp in range(2):
                    for iq in range(2):
                        r = small.tile([65, 512], mybir.dt.float32, tag="r",
                                       name="r")
                        nc.vector.reciprocal(out=r[64:65, :],
                                             in_=pv[hp][iq][64:65, :])
                        bc = psum.tile([64, 512], mybir.dt.float32, tag="pv",
                                       bufs=4, name="bc")
                        nc.tensor.matmul(out=bc, lhsT=ones_sb[64:65, :],
                                         rhs=r[64:65, :], start=True,
                                         stop=True)
                        pvs = small.tile([64, 512], mybir.dt.float32,
                                         tag="pvs", name="pvs")
                        nc.vector.tensor_copy(out=pvs, in_=pv[hp][iq][0:64, :])
                        col = ih * 1024 + iq * 512
                        if hp == 0:
                            nc.vector.tensor_mul(
                                out=att_pair[pair][0:64, col:col + 512],
                                in0=pvs, in1=bc)
                        else:
                            nc.vector.tensor_mul(
                                out=att_odd[pair][:, col:col + 512],
                                in0=pvs, in1=bc)
                            nc.sync.dma_start(
                                out=att_pair[pair][64:128, col:col + 512],
                                in_=att_odd[pair][:, col:col + 512])

            return normalize

        def out_proj():
            # out[s, n] = sum_c att_pair[c].T @ wo_chunk[c]  (K=128 per chunk)
            for st in range(ST):
                po = [psum.tile([P, 512], mybir.dt.float32, tag="pv", bufs=4,
                                name=f"po{nb}") for nb in range(2)]
                for c in range(2):
                    for nb in range(2):
                        nc.tensor.matmul(
                            out=po[nb],
                            lhsT=att_pair[c][:, st * P:(st + 1) * P],
                            rhs=wo_sb[:, c, nb * 512:(nb + 1) * 512],
                            start=(c == 0), stop=(c == 1))
                for nb in range(2):
                    osb = outs_pool.tile([P, 512], mybir.dt.float32,
                                         tag="osb", name="osb")
                    if nb % 2 == 0:
                        nc.vector.tensor_copy(out=osb, in_=po[nb])
                    else:
                        nc.scalar.copy(out=osb, in_=po[nb])
                    nc.sync.dma_start(
                        out=out[st * P:(st + 1) * P,
                                nb * 512:(nb + 1) * 512],
                        in_=osb)

        qk_proj(xq_sb, wq_sb, bq_sb, qT_sb[0], 0)
        qk_proj(xk_sb, wk_sb, bk_sb, kT_sb[0], 0)
        v_proj()
        qk_proj(xq_sb, wq_sb, bq_sb, qT_sb[1], 1)
        qk_proj(xk_sb, wk_sb, bk_sb, kT_sb[1], 1)
        pending = None
        for pair in range(2):
            for ih in range(2):
                pending = attention(pair, ih, pending)
        pending()
        out_proj()

    nc.finalize()
    return nc


def kernel(Q, K, V, Wq, bq, Wk, bk, Wv, bv, Wo, bo):
    from concourse.bass_utils import run_bass_kernel_spmd

    f32 = np.float32
    Q = np.asarray(Q, f32)
    K = np.asarray(K, f32)
    V = np.asarray(V, f32)
    Wq = np.asarray(Wq, f32)
    Wk = np.asarray(Wk, f32)
    Wv = np.asarray(Wv, f32)
    Wo = np.asarray(Wo, f32)
    bq = np.asarray(bq, f32)
    bk = np.asarray(bk, f32)
    bv = np.asarray(bv, f32)
    bo = np.asarray(bo, f32)

    xT = {}
    for b in range(B):
        xT[('q', b)] = np.ascontiguousarray(Q[b].T).astype(BF16)
        xT[('k', b)] = np.ascontiguousarray(K[b].T).astype(BF16)
        xT[('v', b)] = np.ascontiguousarray(V[b].T).astype(BF16)

    in_maps = []
    for c in range(NCORES):
        b, g = c // GROUPS, c % GROUPS
        sl = slice(g * GD, (g + 1) * GD)
        in_maps.append({
            "xqT": xT[('q', b)],
            "xkT": xT[('k', b)],
            "xvT": xT[('v', b)],
            "wq": np.ascontiguousarray(Wq[:, sl]).astype(BF16),
            "wk": np.ascontiguousarray(Wk[:, sl]).astype(BF16),
            "wv": np.ascontiguousarray(Wv[:, sl]).astype(BF16),
            "wo": np.ascontiguousarray(Wo[sl, :]).astype(BF16),
            "bq": np.ascontiguousarray(bq[sl].reshape(GD, 1)),
            "bk": np.ascontiguousarray(bk[sl].reshape(GD, 1)),
        })

    if "nc" not in _cached:
        _cached["nc"] = _build_bass()
    nc = _cached["nc"]

    try:
        res = run_bass_kernel_spmd(nc, in_maps, core_ids=list(range(NCORES)))
    except ModuleNotFoundError:
        # BASS_TRACE set but the axon ntff hook isn't shipped in this
        # container - retry untraced
        os.environ["BASS_NEVER_TRACE"] = "1"
        res = run_bass_kernel_spmd(nc, in_maps, core_ids=list(range(NCORES)))
    if res.exec_time_ns is not None:
        print(f"HW exec time: {res.exec_time_ns} ns")

    bo_eff = (bv @ Wo + bo).astype(f32)
    out = np.zeros((B, S, D), f32)
    for c in range(NCORES):
        b = c // GROUPS
        out[b] += res.results[c]["out"]
    out += bo_eff
    return out



# revision 6
# speedup vs baseline: 1.2879x; 1.2879x over previous
"""Multi-head attention (B=2, S=2048, D=1024, H=16, dk=64) on 8 TRN2 cores.

Sharding: core c -> (batch b = c//4, head-group g = c%4 of 4 heads).
Each core computes q/k/v projections for its 4 heads, full attention for
those heads, and a partial output projection (rows g*256:(g+1)*256 of Wo).
Host pre-transposes/casts inputs to bf16 and sums the partial outputs.

Cost-model-driven layout (PE matmul cost = out-free-size x chunks):
  scoresT[j, i] in PSUM ([128, 2h, 512] per (pair, iq, jt)); exp on ACT
  (one [128, 1024] instr per jt -> probsT bf16; no max-subtract: scores
  ~ N(0,1) after 1/8 scaling).
  PV "orientation B": attn_u[i, e] = sum_j probsT[j,i] * v_aug[j,e] with
  probsT tiles as stationary weights -> 65 rows per matmul instead of 512
  (i on partitions). v_aug = [v_h | ones]; col 64 = softmax denominator.
  Normalize on DVE (reciprocal + broadcast tensor_mul, cast bf16), then
  PE-transpose [128 i, 128 e] blocks back to attnT[e, s] for out-proj.
  out-projT: out[s, n] = sum_c attnT_c[:, s].T @ wo_c (K=128 x2).
Pipelining: k/q/v projections JIT'd into the first attention pass so ACT
starts ~9us in; transpose + out-proj of i-block N-1 pumped as PE filler
into i-block N's ACT-bound slots.  PSUM: sc 2x2 + pv 2 + work 2 = 8 banks.
Host: out[b] = sum_g outT_partial + (bv @ Wo + bo).
"""

import os

import numpy as np
import ml_dtypes

BF16 = ml_dtypes.bfloat16

B, S, D = 2, 2048, 1024
H, DK = 16, 64
P = 128
GROUPS = 4          # head groups (one per core within a batch)
HPG = 4             # heads per group
GD = HPG * DK       # 256, group width
KC = D // P         # 8 contraction chunks
NJT = S // P        # 16 j-tiles
NIQ = S // 512      # 4 i-blocks of 512
NCORES = 8

_cached = {}


def _build_bass():
    import concourse.bass as bass
    import concourse.tile as tile
    from concourse.bacc import Bacc
    from concourse import mybir
    from contextlib import ExitStack

    f32 = mybir.dt.float32
    bf16 = mybir.dt.bfloat16
    Act = mybir.ActivationFunctionType

    nc = Bacc()

    xqT = nc.dram_tensor("xqT", [D, S], bf16, kind="ExternalInput")
    xkT = nc.dram_tensor("xkT", [D, S], bf16, kind="ExternalInput")
    xvT = nc.dram_tensor("xvT", [D, S], bf16, kind="ExternalInput")
    wq = nc.dram_tensor("wq", [D, GD], bf16, kind="ExternalInput")
    wk = nc.dram_tensor("wk", [D, GD], bf16, kind="ExternalInput")
    wv = nc.dram_tensor("wv", [D, GD], bf16, kind="ExternalInput")
    wo = nc.dram_tensor("wo", [GD, D], bf16, kind="ExternalInput")
    bq = nc.dram_tensor("bq", [GD, 1], f32, kind="ExternalInput")
    bk = nc.dram_tensor("bk", [GD, 1], f32, kind="ExternalInput")
    ident = nc.dram_tensor("ident", [P, P], bf16, kind="ExternalInput")
    out = nc.dram_tensor("out", [S, D], f32, kind="ExternalOutput")

    with tile.TileContext(nc) as tc, ExitStack() as ctx:
        singles = ctx.enter_context(tc.tile_pool(name="singles", bufs=1))
        probs_pool = ctx.enter_context(tc.tile_pool(name="probs", bufs=3))
        pre_pool = ctx.enter_context(tc.tile_pool(name="pre", bufs=2))
        small = ctx.enter_context(tc.tile_pool(name="small", bufs=4))
        outs_pool = ctx.enter_context(tc.tile_pool(name="outs", bufs=4))
        psum = ctx.enter_context(tc.tile_pool(name="psum", bufs=1, space="PSUM"))

        # ---- persistent SBUF ----
        wq_sb = singles.tile([P, KC, GD], bf16)
        wk_sb = singles.tile([P, KC, GD], bf16)
        wv_sb = singles.tile([P, KC, GD], bf16)
        wo_sb = singles.tile([P, 2, D], bf16)
        bq_sb = singles.tile([P, 2, 1], f32)
        bk_sb = singles.tile([P, 2, 1], f32)
        ident_sb = singles.tile([P, P], bf16)
        xq_sb = singles.tile([P, KC, S], bf16)
        xk_sb = singles.tile([P, KC, S], bf16)
        xv_sb = singles.tile([P, KC, S], bf16)
        qT = [singles.tile([P, S], bf16, name=f"qT{t}") for t in range(2)]
        kT = [singles.tile([P, S], bf16, name=f"kT{t}") for t in range(2)]
        attT = [singles.tile([P, S], bf16, name=f"attT{t}") for t in range(2)]
        # v_aug per (jt, head): [v | ones]; ones col -> softmax denominator
        v_sb = singles.tile([P, NJT, HPG, 65], bf16)
        nc.vector.memset(v_sb[:, :, :, 64:65], 1.0)

        # ---- DMA emission order = DMA-engine service order ----
        # First-exp critical path: xq i-block 0 (all 8 d-chunks), wq/wk,
        # then xk/xv in 256-col j-blocks (JIT k/v projections chase these).
        def dma_x(dst, src, c0, c1):
            nc.sync.dma_start(
                out=dst[:, :, c0:c1],
                in_=src[:, c0:c1].rearrange("(c p) m -> p c m", p=P))

        dma_x(xq_sb, xqT, 0, 512)
        nc.sync.dma_start(out=wq_sb, in_=wq.rearrange("(c p) m -> p c m", p=P))
        nc.sync.dma_start(out=wk_sb, in_=wk.rearrange("(c p) m -> p c m", p=P))
        nc.sync.dma_start(out=bq_sb, in_=bq.rearrange("(t p) o -> p t o", p=P))
        nc.sync.dma_start(out=bk_sb, in_=bk.rearrange("(t p) o -> p t o", p=P))
        nc.sync.dma_start(out=ident_sb, in_=ident[:, :])
        dma_x(xk_sb, xkT, 0, 256)
        nc.sync.dma_start(out=wv_sb, in_=wv.rearrange("(c p) m -> p c m", p=P))
        dma_x(xv_sb, xvT, 0, 256)
        for m in range(1, 8):
            dma_x(xk_sb, xkT, m * 256, (m + 1) * 256)
            dma_x(xv_sb, xvT, m * 256, (m + 1) * 256)
        for b_ in range(1, 4):
            dma_x(xq_sb, xqT, b_ * 512, (b_ + 1) * 512)
        nc.sync.dma_start(out=wo_sb, in_=wo.rearrange("(c p) n -> p c n", p=P))

        # ---- projection emitters (PSUM "work" slots, 2 rotating banks) ----
        def emit_qproj(p, iqb):
            w = psum.tile([P, 512], f32, tag="work", bufs=2, name="wq_ps")
            for k in range(KC):
                nc.tensor.matmul(
                    out=w,
                    lhsT=wq_sb[:, k, p * P:(p + 1) * P],
                    rhs=xq_sb[:, k, iqb * 512:(iqb + 1) * 512],
                    start=(k == 0), stop=(k == KC - 1))
            nc.vector.tensor_scalar_add(
                out=qT[p][:, iqb * 512:(iqb + 1) * 512], in0=w,
                scalar1=bq_sb[:, p, :])

        def emit_kproj(p, jt):
            w = psum.tile([P, 512], f32, tag="work", bufs=2, name="wk_ps")
            for k in range(KC):
                nc.tensor.matmul(
                    out=w[:, 0:P],
                    lhsT=wk_sb[:, k, p * P:(p + 1) * P],
                    rhs=xk_sb[:, k, jt * P:(jt + 1) * P],
                    start=(k == 0), stop=(k == KC - 1))
            nc.vector.tensor_scalar_add(
                out=kT[p][:, jt * P:(jt + 1) * P], in0=w[:, 0:P],
                scalar1=bk_sb[:, p, :])

        def emit_vproj(p, jt):
            w = psum.tile([P, 512], f32, tag="work", bufs=2, name="wv_ps")
            for k in range(KC):
                nc.tensor.matmul(
                    out=w[:, 0:P],
                    lhsT=xv_sb[:, k, jt * P:(jt + 1) * P],
                    rhs=wv_sb[:, k, p * P:(p + 1) * P],
                    start=(k == 0), stop=(k == KC - 1))
            nc.vector.tensor_copy(
                out=v_sb[:, jt, 2 * p:2 * p + 2, 0:64],
                in_=w[:, 0:P].rearrange("p (h d) -> p h d", h=2))

        # ---- tail units: PE-transpose att_pre -> attT, then out-proj ----
        fillers = []

        def enqueue_tail(iq, att_pre):
            tpp_box = []

            def tp_unit(lo, hi):
                def emit():
                    if not tpp_box:
                        tpp_box.append(psum.tile([P, 8, P], bf16, tag="work",
                                                 bufs=2, name="tpp"))
                    tpp = tpp_box[0]
                    for t in range(lo, hi):
                        pr, it = t // 4, t % 4
                        nc.tensor.matmul(
                            out=tpp[:, t, :],
                            lhsT=att_pre[:, it, pr, :, :].rearrange(
                                "p h d -> p (h d)"),
                            rhs=ident_sb, is_transpose=True,
                            start=True, stop=True)
                    if hi == 8:
                        for pr in range(2):
                            nc.vector.tensor_copy(
                                out=attT[pr][:, iq * 512:(iq + 1) * 512],
                                in_=tpp[:, pr * 4:(pr + 1) * 4, :])
                return emit

            def po_unit(it, nb):
                def emit():
                    po = psum.tile([P, 512], f32, tag="work", bufs=2,
                                   name="po")
                    st = iq * 4 + it
                    for c in range(2):
                        nc.tensor.matmul(
                            out=po,
                            lhsT=attT[c][:, st * P:(st + 1) * P],
                            rhs=wo_sb[:, c, nb * 512:(nb + 1) * 512],
                            start=(c == 0), stop=(c == 1))
                    osb = outs_pool.tile([P, 512], f32, tag="osb", name="osb")
                    nc.vector.tensor_copy(out=osb, in_=po)
                    nc.sync.dma_start(
                        out=out[st * P:(st + 1) * P,
                                nb * 512:(nb + 1) * 512],
                        in_=osb)
                return emit

            fillers.append(tp_unit(0, 4))
            fillers.append(tp_unit(4, 8))
            for it in range(4):
                for nb in range(2):
                    fillers.append(po_unit(it, nb))

        def pump(n):
            for _ in range(n):
                if fillers:
                    fillers.pop(0)()

        # ---- main schedule ----
        emit_qproj(0, 0)
        emit_kproj(0, 0)
        emit_vproj(0, 0)
        for iq in range(NIQ):
            att_pre = pre_pool.tile([P, 4, 2, 2, 64], bf16, tag="pre",
                                    name="att_pre")
            for p in range(2):
                pv = psum.tile([P, 2, 4, P], f32, tag="pv", bufs=1, name="pv")
                for jt in range(NJT):
                    if iq == 0:
                        if jt < NJT - 1:
                            emit_kproj(p, jt + 1)
                            emit_vproj(p, jt + 1)
                        elif p == 0:
                            emit_kproj(1, 0)
                            emit_vproj(1, 0)
                    sc = psum.tile([P, 2, 512], f32, tag="sc", bufs=2,
                                   name="sc")
                    for hp in range(2):
                        nc.tensor.matmul(
                            out=sc[:, hp, :],
                            lhsT=kT[p][hp * 64:(hp + 1) * 64,
                                       jt * P:(jt + 1) * P],
                            rhs=qT[p][hp * 64:(hp + 1) * 64,
                                      iq * 512:(iq + 1) * 512],
                            start=True, stop=True)
                    probs = probs_pool.tile([P, 2, 512], bf16, tag="probs",
                                            name="probs")
                    nc.scalar.activation(out=probs, in_=sc, func=Act.Exp,
                                         scale=0.125)
                    for it in range(4):
                        for hp in range(2):
                            s_ = 2 * it + hp
                            nc.tensor.matmul(
                                out=pv[:, s_ // 4, s_ % 4, 0:65],
                                lhsT=probs[:, hp, it * P:(it + 1) * P],
                                rhs=v_sb[:, jt, 2 * p + hp, :],
                                start=(jt == 0 and s_ % 4 == 0),
                                stop=(jt == NJT - 1 and s_ % 4 == 3))
                    if jt == 10:
                        if p == 0:
                            emit_qproj(1, iq)
                        elif iq < NIQ - 1:
                            emit_qproj(0, iq + 1)
                    pump(1 if iq == 0 else 2)
                # normalize: attn = pv[:, :, :, 0:64] / pv[:, :, :, 64]
                r = small.tile([P, 8], f32, tag="r", name="r")
                nc.vector.reciprocal(
                    out=r, in_=pv[:, :, :, 64:65].rearrange(
                        "p b j o -> p (b j o)"))
                for b_ in range(2):
                    nc.vector.tensor_mul(
                        out=att_pre[:, 2 * b_:2 * b_ + 2, p, :, :],
                        in0=pv[:, b_, :, 0:64].rearrange(
                            "p (i h) d -> p i h d", i=2),
                        in1=r[:, 4 * b_:4 * b_ + 4].rearrange(
                            "p (i h) -> p i h", i=2).to_broadcast(
                            [P, 2, 2, 64]))
            enqueue_tail(iq, att_pre)
        while fillers:
            pump(1)

    nc.finalize()
    return nc


def kernel(Q, K, V, Wq, bq, Wk, bk, Wv, bv, Wo, bo):
    from concourse.bass_utils import run_bass_kernel_spmd

    f32 = np.float32
    Q = np.asarray(Q, f32)
    K = np.asarray(K, f32)
    V = np.asarray(V, f32)
    Wq = np.asarray(Wq, f32)
    Wk = np.asarray(Wk, f32)
    Wv = np.asarray(Wv, f32)
    Wo = np.asarray(Wo, f32)
    bq = np.asarray(bq, f32)
    bk = np.asarray(bk, f32)
    bv = np.asarray(bv, f32)
    bo = np.asarray(bo, f32)

    xT = {}
    for b in range(B):
        xT[('q', b)] = np.ascontiguousarray(Q[b].T).astype(BF16)
        xT[('k', b)] = np.ascontiguousarray(K[b].T).astype(BF16)
        xT[('v', b)] = np.ascontiguousarray(V[b].T).astype(BF16)
    ident_np = np.eye(P, dtype=BF16)

    in_maps = []
    for c in range(NCORES):
        b, g = c // GROUPS, c % GROUPS
        sl = slice(g * GD, (g + 1) * GD)
        in_maps.append({
            "xqT": xT[('q', b)],
            "xkT": xT[('k', b)],
            "xvT": xT[('v', b)],
            "wq": np.ascontiguousarray(Wq[:, sl]).astype(BF16),
            "wk": np.ascontiguousarray(Wk[:, sl]).astype(BF16),
            "wv": np.ascontiguousarray(Wv[:, sl]).astype(BF16),
            "wo": np.ascontiguousarray(Wo[sl, :]).astype(BF16),
            "bq": np.ascontiguousarray(bq[sl].reshape(GD, 1)),
            "bk": np.ascontiguousarray(bk[sl].reshape(GD, 1)),
            "ident": ident_np,
        })

    if "nc" not in _cached:
        _cached["nc"] = _build_bass()
    nc = _cached["nc"]

    try:
        res = run_bass_kernel_spmd(nc, in_maps, core_ids=list(range(NCORES)))
    except ModuleNotFoundError:
        # BASS_TRACE set but the axon ntff hook isn't shipped in this
        # container - retry untraced
        os.environ["BASS_NEVER_TRACE"] = "1"
        res = run_bass_kernel_spmd(nc, in_maps, core_ids=list(range(NCORES)))
    if res.exec_time_ns is not None:
        print(f"HW exec time: {res.exec_time_ns} ns")

    bo_eff = (bv @ Wo + bo).astype(f32)
    out = np.zeros((B, S, D), f32)
    for c in range(NCORES):
        b = c // GROUPS
        out[b] += res.results[c]["out"]
    out += bo_eff
    return out


# revision 11
# speedup vs baseline: 1.3634x; 1.0587x over previous
"""Multi-head attention (B=2, S=2048, D=1024, H=16, dk=64) on 8 TRN2 cores.

Sharding: core c -> (batch b = c//4, head-group g = c%4 of 4 heads).
Each core computes q/k/v projections for its 4 heads, full attention for
those heads, and a partial output projection (rows g*256:(g+1)*256 of Wo).
Host pre-transposes/casts inputs to bf16 and sums the partial outputs.

Cost-model-driven layout (PE matmul cost = out-free-size x chunks):
  scoresT[j, i] in PSUM ([128, 2h, 512] per (pair, iq, jt)); exp on ACT
  (one [128, 1024] instr per jt -> probsT bf16; no max-subtract: scores
  ~ N(0,1) after 1/8 scaling).
  PV "orientation B": attn_u[i, e] = sum_j probsT[j,i] * v_aug[j,e] with
  probsT tiles as stationary weights -> 65 rows per matmul instead of 512
  (i on partitions). v_aug = [v_h | ones]; col 64 = softmax denominator.
  Normalize on DVE (reciprocal + broadcast tensor_mul, cast bf16), then
  PE-transpose [128 i, 128 e] blocks back to attnT[e, s] for out-proj.
  out-projT: out[s, n] = sum_c attnT_c[:, s].T @ wo_c (K=128 x2).
Schedule: flattened (pair, i-block, jt) steps with scores/exp emitted one
step ahead of PV so ACT (the bottleneck: 128 x 1038ns exp) never waits on
PE; k/v projections JIT'd 2 steps ahead inside the first i-block pass;
q-proj split into 2-matmul chunks so no PE burst starves ACT; transpose +
out-proj of i-block N-1 pumped one unit per step into i-block N.
PSUM: sc 2x2 + pv 2 + work 2 = 8 banks.
Host: out[b] = sum_g outT_partial + (bv @ Wo + bo).
"""

import os

import numpy as np
import ml_dtypes

BF16 = ml_dtypes.bfloat16

B, S, D = 2, 2048, 1024
H, DK = 16, 64
P = 128
GROUPS = 4          # head groups (one per core within a batch)
HPG = 4             # heads per group
GD = HPG * DK       # 256, group width
KC = D // P         # 8 contraction chunks
NJT = S // P        # 16 j-tiles
NIQ = S // 512      # 4 i-blocks of 512
NCORES = 8

_cached = {}


def _build_bass():
    import concourse.bass as bass
    import concourse.tile as tile
    from concourse.bacc import Bacc
    from concourse import mybir
    from contextlib import ExitStack

    f32 = mybir.dt.float32
    bf16 = mybir.dt.bfloat16
    Act = mybir.ActivationFunctionType

    nc = Bacc()

    xqT = nc.dram_tensor("xqT", [D, S], bf16, kind="ExternalInput")
    xkT = nc.dram_tensor("xkT", [D, S], bf16, kind="ExternalInput")
    xvT = nc.dram_tensor("xvT", [D, S], bf16, kind="ExternalInput")
    wq = nc.dram_tensor("wq", [D, GD], bf16, kind="ExternalInput")
    wk = nc.dram_tensor("wk", [D, GD], bf16, kind="ExternalInput")
    wv = nc.dram_tensor("wv", [D, GD], bf16, kind="ExternalInput")
    wo = nc.dram_tensor("wo", [GD, D], bf16, kind="ExternalInput")
    bq = nc.dram_tensor("bq", [GD, 1], f32, kind="ExternalInput")
    bk = nc.dram_tensor("bk", [GD, 1], f32, kind="ExternalInput")
    ident = nc.dram_tensor("ident", [P, P], bf16, kind="ExternalInput")
    out = nc.dram_tensor("out", [S, D], bf16, kind="ExternalOutput")

    with tile.TileContext(nc) as tc, ExitStack() as ctx:
        singles = ctx.enter_context(tc.tile_pool(name="singles", bufs=1))
        probs_pool = ctx.enter_context(tc.tile_pool(name="probs", bufs=4))
        pre_pool = ctx.enter_context(tc.tile_pool(name="pre", bufs=2))
        small = ctx.enter_context(tc.tile_pool(name="small", bufs=4))
        outs_pool = ctx.enter_context(tc.tile_pool(name="outs", bufs=4))
        psum = ctx.enter_context(tc.tile_pool(name="psum", bufs=1, space="PSUM"))

        # ---- persistent SBUF ----
        wq_sb = singles.tile([P, KC, GD], bf16)
        wk_sb = singles.tile([P, KC, GD], bf16)
        wv_sb = singles.tile([P, KC, GD], bf16)
        wo_sb = singles.tile([P, 2, D], bf16)
        bq_sb = singles.tile([P, 2, 1], f32)
        bk_sb = singles.tile([P, 2, 1], f32)
        ident_sb = singles.tile([P, P], bf16)
        xq_sb = singles.tile([P, KC, S], bf16)
        xk_sb = singles.tile([P, KC, S], bf16)
        xv_sb = singles.tile([P, KC, S], bf16)
        qT = [singles.tile([P, S], bf16, name=f"qT{t}") for t in range(2)]
        kT = [singles.tile([P, S], bf16, name=f"kT{t}") for t in range(2)]
        attT = [singles.tile([P, S], bf16, name=f"attT{t}") for t in range(2)]
        # v_aug per (jt, head): [v | ones]; ones col -> softmax denominator
        v_sb = singles.tile([P, NJT, HPG, 65], bf16)
        nc.vector.memset(v_sb[:, :, :, 64:65], 1.0)

        # ---- DMA emission order = DMA-engine service order ----
        # First-exp critical path: wq, xq i-block 0 (two 256-col halves),
        # wk, xk j-block 0; then xk/xv 256-col j-blocks chased by the JIT
        # k/v projections during the first i-block pass.
        def dma_x(dst, src, c0, c1):
            nc.sync.dma_start(
                out=dst[:, :, c0:c1],
                in_=src[:, c0:c1].rearrange("(c p) m -> p c m", p=P))

        nc.sync.dma_start(out=wq_sb, in_=wq.rearrange("(c p) m -> p c m", p=P))
        dma_x(xq_sb, xqT, 0, 256)
        nc.sync.dma_start(out=bq_sb, in_=bq.rearrange("(t p) o -> p t o", p=P))
        nc.sync.dma_start(out=bk_sb, in_=bk.rearrange("(t p) o -> p t o", p=P))
        dma_x(xq_sb, xqT, 256, 512)
        nc.sync.dma_start(out=wk_sb, in_=wk.rearrange("(c p) m -> p c m", p=P))
        dma_x(xk_sb, xkT, 0, 256)
        nc.sync.dma_start(out=wv_sb, in_=wv.rearrange("(c p) m -> p c m", p=P))
        dma_x(xv_sb, xvT, 0, 256)
        nc.sync.dma_start(out=ident_sb, in_=ident[:, :])
        for m in range(1, 8):
            dma_x(xk_sb, xkT, m * 256, (m + 1) * 256)
            dma_x(xv_sb, xvT, m * 256, (m + 1) * 256)
        for b_ in range(1, 4):
            dma_x(xq_sb, xqT, b_ * 512, (b_ + 1) * 512)
        nc.sync.dma_start(out=wo_sb, in_=wo.rearrange("(c p) n -> p c n", p=P))

        # ---- projection emitters (PSUM "work" slots, 2 rotating banks) ----
        # q-proj is emitted as 4 independent 128-column parts (each a
        # complete 8-chunk accumulation + bias) so it can spread across
        # steps without holding a work slot across other users.
        def emit_qproj_part(p, iqb, part):
            w = psum.tile([P, 512], f32, tag="work", bufs=2, name="wq_ps")
            c0 = iqb * 512 + part * P
            for k in range(KC):
                nc.tensor.matmul(
                    out=w[:, 0:P],
                    lhsT=wq_sb[:, k, p * P:(p + 1) * P],
                    rhs=xq_sb[:, k, c0:c0 + P],
                    start=(k == 0), stop=(k == KC - 1))
            nc.vector.tensor_scalar_add(
                out=qT[p][:, c0:c0 + P], in0=w[:, 0:P],
                scalar1=bq_sb[:, p, :])

        def emit_qproj(p, iqb):
            for part in range(4):
                emit_qproj_part(p, iqb, part)

        def emit_kproj(p, jt):
            w = psum.tile([P, 512], f32, tag="work", bufs=2, name="wk_ps")
            for k in range(KC):
                nc.tensor.matmul(
                    out=w[:, 0:P],
                    lhsT=wk_sb[:, k, p * P:(p + 1) * P],
                    rhs=xk_sb[:, k, jt * P:(jt + 1) * P],
                    start=(k == 0), stop=(k == KC - 1))
            nc.vector.tensor_scalar_add(
                out=kT[p][:, jt * P:(jt + 1) * P], in0=w[:, 0:P],
                scalar1=bk_sb[:, p, :])

        def emit_vproj(p, jt):
            w = psum.tile([P, 512], f32, tag="work", bufs=2, name="wv_ps")
            for k in range(KC):
                nc.tensor.matmul(
                    out=w[:, 0:P],
                    lhsT=xv_sb[:, k, jt * P:(jt + 1) * P],
                    rhs=wv_sb[:, k, p * P:(p + 1) * P],
                    start=(k == 0), stop=(k == KC - 1))
            nc.vector.tensor_copy(
                out=v_sb[:, jt, 2 * p:2 * p + 2, 0:64],
                in_=w[:, 0:P].rearrange("p (h d) -> p h d", h=2))

        # ---- attention step pieces ----
        def scores_exp(iq, p, jt):
            sc = psum.tile([P, 2, 512], f32, tag="sc", bufs=2, name="sc")
            for hp in range(2):
                nc.tensor.matmul(
                    out=sc[:, hp, :],
                    lhsT=kT[p][hp * 64:(hp + 1) * 64, jt * P:(jt + 1) * P],
                    rhs=qT[p][hp * 64:(hp + 1) * 64,
                              iq * 512:(iq + 1) * 512],
                    start=True, stop=True)
            probs = probs_pool.tile([P, 2, 512], bf16, tag="probs",
                                    name="probs")
            nc.scalar.activation(out=probs, in_=sc, func=Act.Exp, scale=0.125)
            return probs

        def emit_pv(p, jt, probs, pv):
            for it in range(4):
                for hp in range(2):
                    s_ = 2 * it + hp
                    nc.tensor.matmul(
                        out=pv[:, s_ // 4, s_ % 4, 0:65],
                        lhsT=probs[:, hp, it * P:(it + 1) * P],
                        rhs=v_sb[:, jt, 2 * p + hp, :],
                        start=(jt == 0 and s_ % 4 == 0),
                        stop=(jt == NJT - 1 and s_ % 4 == 3))

        def normalize(p, pv, att_pre):
            r = small.tile([P, 8], f32, tag="r", name="r")
            nc.vector.reciprocal(
                out=r, in_=pv[:, :, :, 64:65].rearrange("p b j o -> p (b j o)"))
            for b_ in range(2):
                nc.vector.tensor_mul(
                    out=att_pre[:, 2 * b_:2 * b_ + 2, p, :, :],
                    in0=pv[:, b_, :, 0:64].rearrange(
                        "p (i h) d -> p i h d", i=2),
                    in1=r[:, 4 * b_:4 * b_ + 4].rearrange(
                        "p (i h) -> p i h", i=2).to_broadcast([P, 2, 2, 64]))

        # ---- tail units: PE-transpose att_pre -> attT, then out-proj ----
        # tailA (after pair-0 normalize): transpose pair-0 blocks; tailB
        # (after pair-1): transpose pair-1 blocks + the 8 out-proj units.
        fillers = []

        def tp_unit(iq, att_pre, pr):
            def emit(final):
                tpp = psum.tile([P, 4, P], bf16, tag="work", bufs=2,
                                name="tpp")
                for it in range(4):
                    nc.tensor.matmul(
                        out=tpp[:, it, :],
                        lhsT=att_pre[:, it, pr, :, :].rearrange(
                            "p h d -> p (h d)"),
                        rhs=ident_sb, is_transpose=True,
                        start=True, stop=True)
                nc.vector.tensor_copy(
                    out=attT[pr][:, iq * 512:(iq + 1) * 512],
                    in_=tpp[:, :, :])
            return emit

        osb_box = {}

        def po_unit(iq, it, nb, k):
            def emit(final):
                if final and k % 2 == 0:
                    po = psum.tile([P, 2, 512], f32, tag="sc", bufs=2,
                                   name="po_sc")[:, 0, :]
                else:
                    po = psum.tile([P, 512], f32, tag="work", bufs=2,
                                   name="po")
                st = iq * 4 + it
                for c in range(2):
                    nc.tensor.matmul(
                        out=po,
                        lhsT=attT[c][:, st * P:(st + 1) * P],
                        rhs=wo_sb[:, c, nb * 512:(nb + 1) * 512],
                        start=(c == 0), stop=(c == 1))
                if nb == 0:
                    osb_box[st] = outs_pool.tile([P, 2, 512], bf16,
                                                 tag="osb", bufs=4,
                                                 name="osb")
                osb = osb_box[st]
                # in the final drain ACT is idle: alternate evictions
                if final and k % 2 == 1:
                    nc.scalar.copy(out=osb[:, nb, :], in_=po)
                else:
                    nc.vector.tensor_copy(out=osb[:, nb, :], in_=po)
                if nb == 1:
                    nc.sync.dma_start(
                        out=out[st * P:(st + 1) * P, :],
                        in_=osb_box.pop(st))
            return emit

        def enqueue_tailA(iq, att_pre):
            fillers.append(tp_unit(iq, att_pre, 0))

        def enqueue_tailB(iq, att_pre):
            fillers.append(tp_unit(iq, att_pre, 1))
            k = 0
            for it in range(4):
                for nb in range(2):
                    fillers.append(po_unit(iq, it, nb, k))
                    k += 1

        def pump(n, final=False):
            for _ in range(n):
                if fillers:
                    fillers.pop(0)(final)

        # ---- main schedule ----
        # Flattened (i-block, pair, jt) steps. scores+exp for step g+1 are
        # emitted at iteration g and the PV for step g-1 trails at iteration
        # g, so in PE program order scores(g+2) sits directly behind pv(g):
        # ACT's next exp is never queued behind JIT/pump filler work.
        steps = [(iq, p, jt) for iq in range(NIQ) for p in range(2)
                 for jt in range(NJT)]
        emit_qproj(0, 0)
        emit_kproj(0, 0)
        emit_kproj(0, 1)
        emit_qproj(1, 0)
        emit_kproj(1, 0)
        emit_kproj(1, 1)
        probs_q = {}
        pv = None
        att_pre = None
        for g in range(len(steps) + 1):
            if g >= 1:
                iqp, pp, jtp = steps[g - 1]
                if jtp == 0:
                    pv = psum.tile([P, 2, 4, P], f32, tag="pv", bufs=1,
                                   name="pv")
                    if pp == 0:
                        att_pre = pre_pool.tile([P, 4, 2, 2, 64], bf16,
                                                tag="pre", name="att_pre")
                emit_pv(pp, jtp, probs_q.pop(g - 1), pv)
                if jtp == NJT - 1:
                    normalize(pp, pv, att_pre)
                    if pp == 0:
                        enqueue_tailA(iqp, att_pre)
                    else:
                        enqueue_tailB(iqp, att_pre)
            if g == len(steps):
                break
            iq, p, jt = steps[g]
            if g == 0:
                probs_q[0] = scores_exp(*steps[0])
            if g + 1 < len(steps):
                probs_q[g + 1] = scores_exp(*steps[g + 1])
            if iq == 0:
                emit_vproj(p, jt)
                if g + 2 < 2 * NJT and (g + 2) % NJT >= 2:
                    emit_kproj((g + 2) // NJT, (g + 2) % NJT)
            if 8 <= jt < 12:
                if p == 0:
                    if iq > 0:
                        emit_qproj_part(1, iq, jt - 8)
                elif iq < NIQ - 1:
                    emit_qproj_part(0, iq + 1, jt - 8)
            pump(1)
        while fillers:
            pump(1, final=True)

    nc.finalize()
    return nc


def kernel(Q, K, V, Wq, bq, Wk, bk, Wv, bv, Wo, bo):
    from concourse.bass_utils import run_bass_kernel_spmd

    f32 = np.float32
    Q = np.asarray(Q, f32)
    K = np.asarray(K, f32)
    V = np.asarray(V, f32)
    Wq = np.asarray(Wq, f32)
    Wk = np.asarray(Wk, f32)
    Wv = np.asarray(Wv, f32)
    Wo = np.asarray(Wo, f32)
    bq = np.asarray(bq, f32)
    bk = np.asarray(bk, f32)
    bv = np.asarray(bv, f32)
    bo = np.asarray(bo, f32)

    xT = {}
    for b in range(B):
        xT[('q', b)] = np.ascontiguousarray(Q[b].T).astype(BF16)
        xT[('k', b)] = np.ascontiguousarray(K[b].T).astype(BF16)
        xT[('v', b)] = np.ascontiguousarray(V[b].T).astype(BF16)
    ident_np = np.eye(P, dtype=BF16)

    in_maps = []
    for c in range(NCORES):
        b, g = c // GROUPS, c % GROUPS
        sl = slice(g * GD, (g + 1) * GD)
        in_maps.append({
            "xqT": xT[('q', b)],
            "xkT": xT[('k', b)],
            "xvT": xT[('v', b)],
            "wq": np.ascontiguousarray(Wq[:, sl]).astype(BF16),
            "wk": np.ascontiguousarray(Wk[:, sl]).astype(BF16),
            "wv": np.ascontiguousarray(Wv[:, sl]).astype(BF16),
            "wo": np.ascontiguousarray(Wo[sl, :]).astype(BF16),
            "bq": np.ascontiguousarray(bq[sl].reshape(GD, 1)),
            "bk": np.ascontiguousarray(bk[sl].reshape(GD, 1)),
            "ident": ident_np,
        })

    if "nc" not in _cached:
        _cached["nc"] = _build_bass()
    nc = _cached["nc"]

    try:
        res = run_bass_kernel_spmd(nc, in_maps, core_ids=list(range(NCORES)))
    except ModuleNotFoundError:
        # BASS_TRACE set but the axon ntff hook isn't shipped in this
        # container - retry untraced
        os.environ["BASS_NEVER_TRACE"] = "1"
        res = run_bass_kernel_spmd(nc, in_maps, core_ids=list(range(NCORES)))
    if res.exec_time_ns is not None:
        print(f"HW exec time: {res.exec_time_ns} ns")

    bo_eff = (bv @ Wo + bo).astype(f32)
    out = np.zeros((B, S, D), f32)
    for c in range(NCORES):
        b = c // GROUPS
        out[b] += np.asarray(res.results[c]["out"], f32)
    out += bo_eff
    return out


# revision 13
# speedup vs baseline: 1.3715x; 1.0060x over previous
"""Multi-head attention (B=2, S=2048, D=1024, H=16, dk=64) on 8 TRN2 cores.

Sharding: core c -> (batch b = c//4, head-group g = c%4 of 4 heads).
Each core computes q/k/v projections for its 4 heads, full attention for
those heads, and a partial output projection (rows g*256:(g+1)*256 of Wo).
Host pre-transposes/casts inputs to bf16 and sums the partial outputs.

Cost-model-driven layout (PE matmul cost = out-free-size x chunks):
  scoresT[j, i] in PSUM ([128, 2h, 512] per (pair, iq, jt)); exp on ACT
  (one [128, 1024] instr per jt -> probsT bf16; no max-subtract: scores
  ~ N(0,1) after 1/8 scaling).
  PV "orientation B": attn_u[i, e] = sum_j probsT[j,i] * v_aug[j,e] with
  probsT tiles as stationary weights -> 65 rows per matmul instead of 512
  (i on partitions). v_aug = [v_h | ones]; col 64 = softmax denominator.
  Normalize on DVE (reciprocal + broadcast tensor_mul, cast bf16), then
  PE-transpose [128 i, 128 e] blocks back to attnT[e, s] for out-proj.
  out-projT: out[s, n] = sum_c attnT_c[:, s].T @ wo_c (K=128 x2).
Schedule: flattened (pair, i-block, jt) steps with scores/exp emitted one
step ahead of PV so ACT (the bottleneck: 128 x 1038ns exp) never waits on
PE; k/v projections JIT'd 2 steps ahead inside the first i-block pass;
q-proj split into 2-matmul chunks so no PE burst starves ACT; transpose +
out-proj of i-block N-1 pumped one unit per step into i-block N.
PSUM: sc 2x2 + pv 2 + work 2 = 8 banks.
Host: out[b] = sum_g outT_partial + (bv @ Wo + bo).
"""

import os

import numpy as np
import ml_dtypes

BF16 = ml_dtypes.bfloat16

B, S, D = 2, 2048, 1024
H, DK = 16, 64
P = 128
GROUPS = 4          # head groups (one per core within a batch)
HPG = 4             # heads per group
GD = HPG * DK       # 256, group width
KC = D // P         # 8 contraction chunks
NJT = S // P        # 16 j-tiles
NIQ = S // 512      # 4 i-blocks of 512
NCORES = 8

_cached = {}


def _build_bass():
    import concourse.bass as bass
    import concourse.tile as tile
    from concourse.bacc import Bacc
    from concourse import mybir
    from contextlib import ExitStack

    f32 = mybir.dt.float32
    bf16 = mybir.dt.bfloat16
    Act = mybir.ActivationFunctionType

    nc = Bacc()

    xqT = nc.dram_tensor("xqT", [D, S], bf16, kind="ExternalInput")
    xkT = nc.dram_tensor("xkT", [D, S], bf16, kind="ExternalInput")
    xvT = nc.dram_tensor("xvT", [D, S], bf16, kind="ExternalInput")
    wq = nc.dram_tensor("wq", [D, GD], bf16, kind="ExternalInput")
    wk = nc.dram_tensor("wk", [D, GD], bf16, kind="ExternalInput")
    wv = nc.dram_tensor("wv", [D, GD], bf16, kind="ExternalInput")
    wo = nc.dram_tensor("wo", [GD, D], bf16, kind="ExternalInput")
    bq = nc.dram_tensor("bq", [GD, 1], f32, kind="ExternalInput")
    bk = nc.dram_tensor("bk", [GD, 1], f32, kind="ExternalInput")
    ident = nc.dram_tensor("ident", [P, P], bf16, kind="ExternalInput")
    out = nc.dram_tensor("out", [S, D], bf16, kind="ExternalOutput")

    with tile.TileContext(nc) as tc, ExitStack() as ctx:
        singles = ctx.enter_context(tc.tile_pool(name="singles", bufs=1))
        probs_pool = ctx.enter_context(tc.tile_pool(name="probs", bufs=4))
        pre_pool = ctx.enter_context(tc.tile_pool(name="pre", bufs=2))
        small = ctx.enter_context(tc.tile_pool(name="small", bufs=4))
        outs_pool = ctx.enter_context(tc.tile_pool(name="outs", bufs=4))
        psum = ctx.enter_context(tc.tile_pool(name="psum", bufs=1, space="PSUM"))

        # ---- persistent SBUF ----
        wq_sb = singles.tile([P, KC, GD], bf16)
        wk_sb = singles.tile([P, KC, GD], bf16)
        wv_sb = singles.tile([P, KC, GD], bf16)
        wo_sb = singles.tile([P, 2, D], bf16)
        bq_sb = singles.tile([P, 2, 1], f32)
        bk_sb = singles.tile([P, 2, 1], f32)
        ident_sb = singles.tile([P, P], bf16)
        xq_sb = singles.tile([P, KC, S], bf16)
        xk_sb = singles.tile([P, KC, S], bf16)
        xv_sb = singles.tile([P, KC, S], bf16)
        qT = [singles.tile([P, S], bf16, name=f"qT{t}") for t in range(2)]
        kT = [singles.tile([P, S], bf16, name=f"kT{t}") for t in range(2)]
        attT = [singles.tile([P, S], bf16, name=f"attT{t}") for t in range(2)]
        # v_aug per (jt, head): [v | ones]; ones col -> softmax denominator
        v_sb = singles.tile([P, NJT, HPG, 65], bf16)
        nc.vector.memset(v_sb[:, :, :, 64:65], 1.0)

        # ---- DMA emission order = DMA-engine service order ----
        # First-exp critical path: wq, xq i-block 0 (two 256-col halves),
        # wk, xk j-block 0; then xk/xv 256-col j-blocks chased by the JIT
        # k/v projections during the first i-block pass.
        def dma_x(dst, src, c0, c1):
            nc.sync.dma_start(
                out=dst[:, :, c0:c1],
                in_=src[:, c0:c1].rearrange("(c p) m -> p c m", p=P))

        nc.sync.dma_start(out=wq_sb, in_=wq.rearrange("(c p) m -> p c m", p=P))
        dma_x(xq_sb, xqT, 0, 256)
        nc.sync.dma_start(out=bq_sb, in_=bq.rearrange("(t p) o -> p t o", p=P))
        nc.sync.dma_start(out=bk_sb, in_=bk.rearrange("(t p) o -> p t o", p=P))
        dma_x(xq_sb, xqT, 256, 512)
        nc.sync.dma_start(out=wk_sb, in_=wk.rearrange("(c p) m -> p c m", p=P))
        dma_x(xk_sb, xkT, 0, 256)
        nc.sync.dma_start(out=wv_sb, in_=wv.rearrange("(c p) m -> p c m", p=P))
        dma_x(xv_sb, xvT, 0, 256)
        nc.sync.dma_start(out=ident_sb, in_=ident[:, :])
        for m in range(1, 8):
            dma_x(xk_sb, xkT, m * 256, (m + 1) * 256)
            dma_x(xv_sb, xvT, m * 256, (m + 1) * 256)
        for b_ in range(1, 4):
            dma_x(xq_sb, xqT, b_ * 512, (b_ + 1) * 512)
        nc.sync.dma_start(out=wo_sb, in_=wo.rearrange("(c p) n -> p c n", p=P))

        # ---- projection emitters (PSUM "work" slots, 2 rotating banks) ----
        # q-proj is emitted as 4 independent 128-column parts (each a
        # complete 8-chunk accumulation + bias) so it can spread across
        # steps without holding a work slot across other users.
        def emit_qproj_part(p, iqb, part):
            w = psum.tile([P, 512], f32, tag="work", bufs=2, name="wq_ps")
            c0 = iqb * 512 + part * P
            for k in range(KC):
                nc.tensor.matmul(
                    out=w[:, 0:P],
                    lhsT=wq_sb[:, k, p * P:(p + 1) * P],
                    rhs=xq_sb[:, k, c0:c0 + P],
                    start=(k == 0), stop=(k == KC - 1))
            nc.vector.tensor_scalar_add(
                out=qT[p][:, c0:c0 + P], in0=w[:, 0:P],
                scalar1=bq_sb[:, p, :])

        def emit_qproj(p, iqb):
            for part in range(4):
                emit_qproj_part(p, iqb, part)

        def emit_kproj(p, jt):
            w = psum.tile([P, 512], f32, tag="work", bufs=2, name="wk_ps")
            for k in range(KC):
                nc.tensor.matmul(
                    out=w[:, 0:P],
                    lhsT=wk_sb[:, k, p * P:(p + 1) * P],
                    rhs=xk_sb[:, k, jt * P:(jt + 1) * P],
                    start=(k == 0), stop=(k == KC - 1))
            nc.vector.tensor_scalar_add(
                out=kT[p][:, jt * P:(jt + 1) * P], in0=w[:, 0:P],
                scalar1=bk_sb[:, p, :])

        def emit_vproj(p, jt):
            w = psum.tile([P, 512], f32, tag="work", bufs=2, name="wv_ps")
            for k in range(KC):
                nc.tensor.matmul(
                    out=w[:, 0:P],
                    lhsT=xv_sb[:, k, jt * P:(jt + 1) * P],
                    rhs=wv_sb[:, k, p * P:(p + 1) * P],
                    start=(k == 0), stop=(k == KC - 1))
            nc.vector.tensor_copy(
                out=v_sb[:, jt, 2 * p:2 * p + 2, 0:64],
                in_=w[:, 0:P].rearrange("p (h d) -> p h d", h=2))

        # ---- attention step pieces ----
        def scores_exp(iq, p, jt):
            sc = psum.tile([P, 2, 512], f32, tag="sc", bufs=2, name="sc")
            for hp in range(2):
                nc.tensor.matmul(
                    out=sc[:, hp, :],
                    lhsT=kT[p][hp * 64:(hp + 1) * 64, jt * P:(jt + 1) * P],
                    rhs=qT[p][hp * 64:(hp + 1) * 64,
                              iq * 512:(iq + 1) * 512],
                    start=True, stop=True)
            probs = probs_pool.tile([P, 2, 512], bf16, tag="probs",
                                    name="probs")
            nc.scalar.activation(out=probs, in_=sc, func=Act.Exp, scale=0.125)
            return probs

        def emit_pv(p, jt, probs, pv):
            for it in range(4):
                for hp in range(2):
                    s_ = 2 * it + hp
                    nc.tensor.matmul(
                        out=pv[:, s_ // 4, s_ % 4, 0:65],
                        lhsT=probs[:, hp, it * P:(it + 1) * P],
                        rhs=v_sb[:, jt, 2 * p + hp, :],
                        start=(jt == 0 and s_ % 4 == 0),
                        stop=(jt == NJT - 1 and s_ % 4 == 3))

        def normalize(p, pv, att_pre):
            r = small.tile([P, 8], f32, tag="r", name="r")
            nc.vector.reciprocal(
                out=r, in_=pv[:, :, :, 64:65].rearrange("p b j o -> p (b j o)"))
            nc.vector.tensor_mul(
                out=att_pre[:, :, p, :, :],
                in0=pv[:, :, :, 0:64].rearrange(
                    "p b (i h) d -> p (b i) h d", i=2),
                in1=r.rearrange("p (x h) -> p x h", h=2).to_broadcast(
                    [P, 4, 2, 64]))

        # ---- tail units: PE-transpose att_pre -> attT, then out-proj ----
        # tailA (after pair-0 normalize): transpose pair-0 blocks; tailB
        # (after pair-1): transpose pair-1 blocks + the 8 out-proj units.
        fillers = []

        def tp_unit(iq, att_pre, pr):
            def emit(final):
                tpp = psum.tile([P, 4, P], bf16, tag="work", bufs=2,
                                name="tpp")
                for it in range(4):
                    nc.tensor.matmul(
                        out=tpp[:, it, :],
                        lhsT=att_pre[:, it, pr, :, :].rearrange(
                            "p h d -> p (h d)"),
                        rhs=ident_sb, is_transpose=True,
                        start=True, stop=True)
                nc.vector.tensor_copy(
                    out=attT[pr][:, iq * 512:(iq + 1) * 512],
                    in_=tpp[:, :, :])
            return emit

        osb_box = {}

        def po_unit(iq, it, nb, k):
            def emit(final):
                if final and k % 2 == 0:
                    po = psum.tile([P, 2, 512], f32, tag="sc", bufs=2,
                                   name="po_sc")[:, 0, :]
                else:
                    po = psum.tile([P, 512], f32, tag="work", bufs=2,
                                   name="po")
                st = iq * 4 + it
                for c in range(2):
                    nc.tensor.matmul(
                        out=po,
                        lhsT=attT[c][:, st * P:(st + 1) * P],
                        rhs=wo_sb[:, c, nb * 512:(nb + 1) * 512],
                        start=(c == 0), stop=(c == 1))
                if nb == 0:
                    osb_box[st] = outs_pool.tile([P, 2, 512], bf16,
                                                 tag="osb", bufs=4,
                                                 name="osb")
                osb = osb_box[st]
                # in the final drain ACT is idle: alternate evictions
                if final and k % 2 == 1:
                    nc.scalar.copy(out=osb[:, nb, :], in_=po)
                else:
                    nc.vector.tensor_copy(out=osb[:, nb, :], in_=po)
                if nb == 1:
                    nc.sync.dma_start(
                        out=out[st * P:(st + 1) * P, :],
                        in_=osb_box.pop(st))
            return emit

        def enqueue_tailA(iq, att_pre):
            fillers.append(tp_unit(iq, att_pre, 0))

        def enqueue_tailB(iq, att_pre):
            fillers.append(tp_unit(iq, att_pre, 1))
            k = 0
            for it in range(4):
                for nb in range(2):
                    fillers.append(po_unit(iq, it, nb, k))
                    k += 1

        def pump(n, final=False):
            for _ in range(n):
                if fillers:
                    fillers.pop(0)(final)

        # ---- main schedule ----
        # Flattened (i-block, pair, jt) steps. scores+exp for step g+1 are
        # emitted at iteration g and the PV for step g-1 trails at iteration
        # g, so in PE program order scores(g+2) sits directly behind pv(g):
        # ACT's next exp is never queued behind JIT/pump filler work.
        steps = [(iq, p, jt) for iq in range(NIQ) for p in range(2)
                 for jt in range(NJT)]
        emit_qproj(0, 0)
        emit_kproj(0, 0)
        emit_kproj(0, 1)
        emit_qproj(1, 0)
        emit_kproj(1, 0)
        emit_kproj(1, 1)
        probs_q = {}
        pv = None
        att_pre = None
        for g in range(len(steps) + 1):
            if g >= 1:
                iqp, pp, jtp = steps[g - 1]
                if jtp == 0:
                    pv = psum.tile([P, 2, 4, P], f32, tag="pv", bufs=1,
                                   name="pv")
                    if pp == 0:
                        att_pre = pre_pool.tile([P, 4, 2, 2, 64], bf16,
                                                tag="pre", name="att_pre")
                emit_pv(pp, jtp, probs_q.pop(g - 1), pv)
                if jtp == NJT - 1:
                    normalize(pp, pv, att_pre)
                    if pp == 0:
                        enqueue_tailA(iqp, att_pre)
                    else:
                        enqueue_tailB(iqp, att_pre)
            if g == len(steps):
                break
            iq, p, jt = steps[g]
            if g == 0:
                probs_q[0] = scores_exp(*steps[0])
            if g + 1 < len(steps):
                probs_q[g + 1] = scores_exp(*steps[g + 1])
            if iq == 0:
                emit_vproj(p, jt)
                if g + 2 < 2 * NJT and (g + 2) % NJT >= 2:
                    emit_kproj((g + 2) // NJT, (g + 2) % NJT)
            if 8 <= jt < 16 and jt % 2 == 0:
                if p == 0:
                    if iq > 0:
                        emit_qproj_part(1, iq, (jt - 8) // 2)
                elif iq < NIQ - 1:
                    emit_qproj_part(0, iq + 1, (jt - 8) // 2)
            pump(1)
        while fillers:
            pump(1, final=True)

    nc.finalize()
    return nc


def kernel(Q, K, V, Wq, bq, Wk, bk, Wv, bv, Wo, bo):
    from concourse.bass_utils import run_bass_kernel_spmd

    f32 = np.float32
    Q = np.asarray(Q, f32)
    K = np.asarray(K, f32)
    V = np.asarray(V, f32)
    Wq = np.asarray(Wq, f32)
    Wk = np.asarray(Wk, f32)
    Wv = np.asarray(Wv, f32)
    Wo = np.asarray(Wo, f32)
    bq = np.asarray(bq, f32)
    bk = np.asarray(bk, f32)
    bv = np.asarray(bv, f32)
    bo = np.asarray(bo, f32)

    xT = {}
    for b in range(B):
        xT[('q', b)] = np.ascontiguousarray(Q[b].T).astype(BF16)
        xT[('k', b)] = np.ascontiguousarray(K[b].T).astype(BF16)
        xT[('v', b)] = np.ascontiguousarray(V[b].T).astype(BF16)
    ident_np = np.eye(P, dtype=BF16)

    in_maps = []
    for c in range(NCORES):
        b, g = c // GROUPS, c % GROUPS
        sl = slice(g * GD, (g + 1) * GD)
        in_maps.append({
            "xqT": xT[('q', b)],
            "xkT": xT[('k', b)],
            "xvT": xT[('v', b)],
            "wq": np.ascontiguousarray(Wq[:, sl]).astype(BF16),
            "wk": np.ascontiguousarray(Wk[:, sl]).astype(BF16),
            "wv": np.ascontiguousarray(Wv[:, sl]).astype(BF16),
            "wo": np.ascontiguousarray(Wo[sl, :]).astype(BF16),
            "bq": np.ascontiguousarray(bq[sl].reshape(GD, 1)),
            "bk": np.ascontiguousarray(bk[sl].reshape(GD, 1)),
            "ident": ident_np,
        })

    if "nc" not in _cached:
        _cached["nc"] = _build_bass()
    nc = _cached["nc"]

    try:
        res = run_bass_kernel_spmd(nc, in_maps, core_ids=list(range(NCORES)))
    except ModuleNotFoundError:
        # BASS_TRACE set but the axon ntff hook isn't shipped in this
        # container - retry untraced
        os.environ["BASS_NEVER_TRACE"] = "1"
        res = run_bass_kernel_spmd(nc, in_maps, core_ids=list(range(NCORES)))
    if res.exec_time_ns is not None:
        print(f"HW exec time: {res.exec_time_ns} ns")

    bo_eff = (bv @ Wo + bo).astype(f32)
    out = np.zeros((B, S, D), f32)
    for c in range(NCORES):
        b = c // GROUPS
        out[b] += np.asarray(res.results[c]["out"], f32)
    out += bo_eff
    return out


# revision 24
# speedup vs baseline: 1.4156x; 1.0321x over previous
"""Multi-head attention (B=2, S=2048, D=1024, H=16, dk=64) on 8 TRN2 cores.

Sharding: core c -> (batch b = c//4, head-group g = c%4 of 4 heads).
Each core computes q/k/v projections for its 4 heads, full attention for
those heads, and a partial output projection (rows g*256:(g+1)*256 of Wo).
Host pre-transposes/casts inputs to bf16 and sums the partial outputs.

Cost-model-driven layout (PE matmul cost = out-free-size x chunks):
  scoresT[j, i] in PSUM ([128, 2h, 512] per (pair, iq, jt)); exp on ACT
  (one [128, 1024] instr per jt -> probsT bf16; no max-subtract: scores
  ~ N(0,1) after 1/8 scaling).
  PV "orientation B": attn_u[i, e] = sum_j probsT[j,i] * v_aug[j,e] with
  probsT tiles as stationary weights -> 65 rows per matmul instead of 512
  (i on partitions). v_aug = [v_h | ones]; col 64 = softmax denominator.
  Normalize on DVE (reciprocal + broadcast tensor_mul, cast bf16), then
  PE-transpose [128 i, 128 e] blocks back to attnT[e, s] for out-proj.
  out-projT: out[s, n] = sum_c attnT_c[:, s].T @ wo_c (K=128 x2).
Schedule: flattened (pair, i-block, jt) steps with scores/exp emitted one
step ahead of PV so ACT (the bottleneck: 128 x 1038ns exp) never waits on
PE; k/v projections JIT'd 2 steps ahead inside the first i-block pass;
q-proj split into 2-matmul chunks so no PE burst starves ACT; transpose +
out-proj of i-block N-1 pumped one unit per step into i-block N.
PSUM: sc 2x2 + pv 2 + work 2 = 8 banks.
Host: out[b] = sum_g outT_partial + (bv @ Wo + bo).
"""

import os

import numpy as np
import ml_dtypes

BF16 = ml_dtypes.bfloat16

B, S, D = 2, 2048, 1024
H, DK = 16, 64
P = 128
GROUPS = 4          # head groups (one per core within a batch)
HPG = 4             # heads per group
GD = HPG * DK       # 256, group width
KC = D // P         # 8 contraction chunks
NJT = S // P        # 16 j-tiles
NIQ = S // 512      # 4 i-blocks of 512
NCORES = 8

_cached = {}


def _build_bass():
    import concourse.bass as bass
    import concourse.tile as tile
    from concourse.bacc import Bacc
    from concourse import mybir
    from contextlib import ExitStack

    f32 = mybir.dt.float32
    bf16 = mybir.dt.bfloat16
    Act = mybir.ActivationFunctionType

    nc = Bacc()

    xqT = nc.dram_tensor("xqT", [D, S], bf16, kind="ExternalInput")
    xkT = nc.dram_tensor("xkT", [D, S], bf16, kind="ExternalInput")
    xvT = nc.dram_tensor("xvT", [D, S], bf16, kind="ExternalInput")
    wq = nc.dram_tensor("wq", [D, GD], bf16, kind="ExternalInput")
    wk = nc.dram_tensor("wk", [D, GD], bf16, kind="ExternalInput")
    wv = nc.dram_tensor("wv", [D, GD], bf16, kind="ExternalInput")
    wo = nc.dram_tensor("wo", [GD, D], bf16, kind="ExternalInput")
    bq = nc.dram_tensor("bq", [GD, 1], f32, kind="ExternalInput")
    bk = nc.dram_tensor("bk", [GD, 1], f32, kind="ExternalInput")
    ident = nc.dram_tensor("ident", [P, P], bf16, kind="ExternalInput")
    out = nc.dram_tensor("out", [S, D], bf16, kind="ExternalOutput")

    with tile.TileContext(nc) as tc, ExitStack() as ctx:
        singles = ctx.enter_context(tc.tile_pool(name="singles", bufs=1))
        probs_pool = ctx.enter_context(tc.tile_pool(name="probs", bufs=4))
        pre_pool = ctx.enter_context(tc.tile_pool(name="pre", bufs=4))
        small = ctx.enter_context(tc.tile_pool(name="small", bufs=4))
        outs_pool = ctx.enter_context(tc.tile_pool(name="outs", bufs=4))
        psum = ctx.enter_context(tc.tile_pool(name="psum", bufs=1, space="PSUM"))

        # ---- persistent SBUF ----
        wq_sb = singles.tile([P, KC, GD], bf16)
        wk_sb = singles.tile([P, KC, GD], bf16)
        wv_sb = singles.tile([P, KC, GD], bf16)
        wo_sb = singles.tile([P, 2, D], bf16)
        bq_sb = singles.tile([P, 2, 1], f32)
        bk_sb = singles.tile([P, 2, 1], f32)
        ident_sb = singles.tile([P, P], bf16)
        xq_sb = singles.tile([P, KC, S], bf16)
        xk_sb = singles.tile([P, KC, S], bf16)
        xv_sb = singles.tile([P, KC, S], bf16)
        qT = [singles.tile([P, S], bf16, name=f"qT{t}") for t in range(2)]
        kT = [singles.tile([P, S], bf16, name=f"kT{t}") for t in range(2)]
        attT = [singles.tile([P, S], bf16, name=f"attT{t}") for t in range(2)]
        # v_aug per (jt, head): [v | ones]; ones col -> softmax denominator
        v_sb = singles.tile([P, NJT, HPG, 65], bf16)
        nc.vector.memset(v_sb[:, :, :, 64:65], 1.0)

        # ---- DMA emission order = DMA-engine service order ----
        # First-exp critical path: wq, xq i-block 0 (two 256-col halves),
        # wk, xk j-block 0; then xk/xv 256-col j-blocks chased by the JIT
        # k/v projections during the first i-block pass.
        def dma_x(dst, src, c0, c1):
            nc.sync.dma_start(
                out=dst[:, :, c0:c1],
                in_=src[:, c0:c1].rearrange("(c p) m -> p c m", p=P))

        nc.sync.dma_start(out=wq_sb, in_=wq.rearrange("(c p) m -> p c m", p=P))
        dma_x(xq_sb, xqT, 0, 256)
        nc.sync.dma_start(out=bq_sb, in_=bq.rearrange("(t p) o -> p t o", p=P))
        nc.sync.dma_start(out=bk_sb, in_=bk.rearrange("(t p) o -> p t o", p=P))
        dma_x(xq_sb, xqT, 256, 512)
        nc.sync.dma_start(out=wk_sb, in_=wk.rearrange("(c p) m -> p c m", p=P))
        dma_x(xk_sb, xkT, 0, 256)
        nc.sync.dma_start(out=wv_sb, in_=wv.rearrange("(c p) m -> p c m", p=P))
        dma_x(xv_sb, xvT, 0, 256)
        nc.sync.dma_start(out=ident_sb, in_=ident[:, :])
        for m in range(1, 8):
            dma_x(xk_sb, xkT, m * 256, (m + 1) * 256)
            dma_x(xv_sb, xvT, m * 256, (m + 1) * 256)
            if m == 4:
                dma_x(xq_sb, xqT, 512, 1024)
        for b_ in range(2, 4):
            dma_x(xq_sb, xqT, b_ * 512, (b_ + 1) * 512)
        nc.sync.dma_start(out=wo_sb, in_=wo.rearrange("(c p) n -> p c n", p=P))

        # ---- projection emitters (PSUM "work" slots, 2 rotating banks) ----
        # q-proj is emitted as 4 independent 128-column parts (each a
        # complete 8-chunk accumulation + bias) so it can spread across
        # steps without holding a work slot across other users.
        def emit_qproj_part(p, iqb, part):
            w = psum.tile([P, 512], f32, tag="work", bufs=2, name="wq_ps")
            c0 = iqb * 512 + part * P
            for k in range(KC):
                nc.tensor.matmul(
                    out=w[:, 0:P],
                    lhsT=wq_sb[:, k, p * P:(p + 1) * P],
                    rhs=xq_sb[:, k, c0:c0 + P],
                    start=(k == 0), stop=(k == KC - 1))
            nc.vector.tensor_scalar_add(
                out=qT[p][:, c0:c0 + P], in0=w[:, 0:P],
                scalar1=bq_sb[:, p, :])

        def emit_qproj(p, iqb):
            for part in range(4):
                emit_qproj_part(p, iqb, part)

        def emit_kproj(p, jt, w=None, stop=True):
            if w is None:
                w = psum.tile([P, 512], f32, tag="work", bufs=2, name="wk_ps")
            for k in range(KC):
                nc.tensor.matmul(
                    out=w[:, 0:P],
                    lhsT=wk_sb[:, k, p * P:(p + 1) * P],
                    rhs=xk_sb[:, k, jt * P:(jt + 1) * P],
                    start=(k == 0), stop=(stop and k == KC - 1))
            nc.vector.tensor_scalar_add(
                out=kT[p][:, jt * P:(jt + 1) * P], in0=w[:, 0:P],
                scalar1=bk_sb[:, p, :])

        def emit_vproj(p, jt, w=None, start=True):
            if w is None:
                w = psum.tile([P, 512], f32, tag="work", bufs=2, name="wv_ps")
            for k in range(KC):
                nc.tensor.matmul(
                    out=w[:, P:2 * P] if not start else w[:, 0:P],
                    lhsT=xv_sb[:, k, jt * P:(jt + 1) * P],
                    rhs=wv_sb[:, k, p * P:(p + 1) * P],
                    start=(start and k == 0), stop=(k == KC - 1))
            src_ = w[:, P:2 * P] if not start else w[:, 0:P]
            nc.vector.tensor_copy(
                out=v_sb[:, jt, 2 * p:2 * p + 2, 0:64],
                in_=src_.rearrange("p (h d) -> p h d", h=2))

        def emit_kvproj(p, jt):
            # one work tile, one accumulation group: kproj in [:, 0:128]
            # (opens the zero region), vproj in [:, 128:256] (first-touch
            # overwrite inside the pending region, closes the group)
            w = psum.tile([P, 512], f32, tag="work", bufs=2, name="wkv_ps")
            emit_kproj(p, jt, w=w, stop=False)
            emit_vproj(p, jt, w=w, start=False)

        # ---- attention step pieces ----
        def scores_exp(iq, p, jt):
            sc = psum.tile([P, 2, 512], f32, tag="sc", bufs=2, name="sc")
            for hp in range(2):
                nc.tensor.matmul(
                    out=sc[:, hp, :],
                    lhsT=kT[p][hp * 64:(hp + 1) * 64, jt * P:(jt + 1) * P],
                    rhs=qT[p][hp * 64:(hp + 1) * 64,
                              iq * 512:(iq + 1) * 512],
                    start=True, stop=True)
            probs = probs_pool.tile([P, 2, 512], bf16, tag="probs",
                                    name="probs")
            nc.scalar.activation(out=probs, in_=sc, func=Act.Exp, scale=0.125)
            return probs

        def emit_pv(p, jt, probs, pv):
            for it in range(4):
                for hp in range(2):
                    s_ = 2 * it + hp
                    nc.tensor.matmul(
                        out=pv[:, s_ // 4, s_ % 4, 0:65],
                        lhsT=probs[:, hp, it * P:(it + 1) * P],
                        rhs=v_sb[:, jt, 2 * p + hp, :],
                        start=(jt == 0 and s_ % 4 == 0),
                        stop=(jt == NJT - 1 and s_ % 4 == 3))

        def normalize(p, pv, att_pre):
            r = small.tile([P, 8], f32, tag="r", name="r")
            nc.vector.reciprocal(
                out=r, in_=pv[:, :, :, 64:65].rearrange("p b j o -> p (b j o)"))
            nc.vector.tensor_mul(
                out=att_pre[:, :, p, :, :],
                in0=pv[:, :, :, 0:64].rearrange(
                    "p b (i h) d -> p (b i) h d", i=2),
                in1=r.rearrange("p (x h) -> p x h", h=2).to_broadcast(
                    [P, 4, 2, 64]))

        # ---- tail units: PE-transpose att_pre -> attT, then out-proj ----
        # tailA (after pair-0 normalize): transpose pair-0 blocks; tailB
        # (after pair-1): transpose pair-1 blocks + the 8 out-proj units.
        fillers = []

        def tp_unit(iq, att_pre, pr):
            def emit(final):
                tpp = psum.tile([P, 4, P], bf16, tag="work", bufs=2,
                                name="tpp")
                for it in range(4):
                    nc.tensor.matmul(
                        out=tpp[:, it, :],
                        lhsT=att_pre[:, it, pr, :, :].rearrange(
                            "p h d -> p (h d)"),
                        rhs=ident_sb, is_transpose=True,
                        start=True, stop=True)
                nc.vector.tensor_copy(
                    out=attT[pr][:, iq * 512:(iq + 1) * 512],
                    in_=tpp[:, :, :])
            return emit

        osb_box = {}

        def po_unit(iq, it, nb, k):
            def emit(final):
                if final and k % 2 == 0:
                    po = psum.tile([P, 2, 512], f32, tag="sc", bufs=2,
                                   name="po_sc")[:, 0, :]
                else:
                    po = psum.tile([P, 512], f32, tag="work", bufs=2,
                                   name="po")
                st = iq * 4 + it
                for c in range(2):
                    nc.tensor.matmul(
                        out=po,
                        lhsT=attT[c][:, st * P:(st + 1) * P],
                        rhs=wo_sb[:, c, nb * 512:(nb + 1) * 512],
                        start=(c == 0), stop=(c == 1))
                if nb == 0:
                    osb_box[st] = outs_pool.tile([P, 2, 512], bf16,
                                                 tag="osb", bufs=4,
                                                 name="osb")
                osb = osb_box[st]
                # in the final drain ACT is idle: alternate evictions
                if final and k % 2 == 1:
                    nc.scalar.copy(out=osb[:, nb, :], in_=po)
                else:
                    nc.vector.tensor_copy(out=osb[:, nb, :], in_=po)
                if nb == 1:
                    nc.sync.dma_start(
                        out=out[st * P:(st + 1) * P, :],
                        in_=osb_box.pop(st))
            return emit

        def enqueue_tailA(iq, att_pre):
            fillers.append((0, tp_unit(iq, att_pre, 0)))

        def enqueue_tailB(iq, att_pre):
            fillers.append((0, tp_unit(iq, att_pre, 1)))
            k = 0
            for it in range(4):
                for nb in range(2):
                    fillers.append((0, po_unit(iq, it, nb, k)))
                    k += 1

        def pump(n, final=False, g=10 ** 9):
            for _ in range(n):
                if fillers and fillers[0][0] <= g:
                    fillers.pop(0)[1](final)

        # ---- main schedule ----
        # Pair-major pass order: all four i-blocks for pair 0, then pair 1.
        # Pair 1's k/v projections and the later q-proj parts become filler
        # units drained into the ACT-bound slack of passes 2-8 (min_g gates
        # a unit until its DMA block has landed, so a pumped unit never
        # head-blocks the PE queue). scores+exp for step g+1 are emitted at
        # iteration g and the PV for step g-1 trails at iteration g, so in
        # PE program order scores(g+2) sits directly behind pv(g).
        passes = [(iq, 0) for iq in range(NIQ)] + \
                 [(iq, 1) for iq in range(NIQ)]
        steps = [(iq, p, jt) for (iq, p) in passes for jt in range(NJT)]
        emit_qproj(0, 0)
        emit_kproj(0, 0)
        emit_kproj(0, 1)
        emit_qproj(1, 0)
        emit_kproj(1, 0)
        emit_kproj(1, 1)

        def qproj_unit(p_, iqb, part):
            return lambda final: emit_qproj_part(p_, iqb, part)

        def kproj_unit(p_, jt_):
            return lambda final: emit_kproj(p_, jt_)

        def vproj_unit(p_, jt_):
            return lambda final: emit_vproj(p_, jt_)

        for part in range(4):
            fillers.append((10, qproj_unit(0, 1, part)))
        for part in range(4):
            fillers.append((18, qproj_unit(0, 2, part)))
        for part in range(4):
            fillers.append((20, qproj_unit(0, 3, part)))
        fillers.append((20, vproj_unit(1, 0)))
        fillers.append((20, vproj_unit(1, 1)))
        for jt_ in range(2, NJT):
            fillers.append((max(20, jt_ + 2), kproj_unit(1, jt_)))
            fillers.append((max(20, jt_ + 2), vproj_unit(1, jt_)))
        for b_ in range(1, 4):
            for part in range(4):
                fillers.append((40, qproj_unit(1, b_, part)))

        probs_q = {}
        pv = None
        att_pre_map = {}
        pending_norm = None
        for g in range(len(steps) + 1):
            if g >= 1:
                iqp, pp, jtp = steps[g - 1]
                if jtp == 0:
                    pv = psum.tile([P, 2, 4, P], f32, tag="pv", bufs=1,
                                   name="pv")
                emit_pv(pp, jtp, probs_q.pop(g - 1), pv)
                if jtp == NJT - 1:
                    pending_norm = (pp, pv, iqp)
            if g == len(steps):
                if pending_norm is not None:
                    pp, pvn, iqn = pending_norm
                    normalize(pp, pvn, att_pre_map[iqn])
                    enqueue_tailB(iqn, att_pre_map[iqn])
                break
            iq, p, jt = steps[g]
            if g == 0:
                probs_q[0] = scores_exp(*steps[0])
            if g + 1 < len(steps):
                probs_q[g + 1] = scores_exp(*steps[g + 1])
            if iq == 0 and p == 0:
                emit_vproj(0, jt)
                if jt + 2 < NJT:
                    emit_kproj(0, jt + 2)
            if pending_norm is not None:
                pp, pvn, iqn = pending_norm
                if pp == 0:
                    att_pre_map[iqn] = pre_pool.tile(
                        [P, 4, 2, 2, 64], bf16, tag="pre", name="att_pre")
                normalize(pp, pvn, att_pre_map[iqn])
                if pp == 0:
                    enqueue_tailA(iqn, att_pre_map[iqn])
                else:
                    enqueue_tailB(iqn, att_pre_map[iqn])
                pending_norm = None
            if 2 <= jt < 14:
                pump(2 if jt % 2 == 0 else 1, g=g)
        while fillers:
            pump(1, final=True)

    nc.finalize()
    return nc


def kernel(Q, K, V, Wq, bq, Wk, bk, Wv, bv, Wo, bo):
    from concourse.bass_utils import run_bass_kernel_spmd

    f32 = np.float32
    Q = np.asarray(Q, f32)
    K = np.asarray(K, f32)
    V = np.asarray(V, f32)
    Wq = np.asarray(Wq, f32)
    Wk = np.asarray(Wk, f32)
    Wv = np.asarray(Wv, f32)
    Wo = np.asarray(Wo, f32)
    bq = np.asarray(bq, f32)
    bk = np.asarray(bk, f32)
    bv = np.asarray(bv, f32)
    bo = np.asarray(bo, f32)

    xT = {}
    for b in range(B):
        xT[('q', b)] = np.ascontiguousarray(Q[b].T).astype(BF16)
        xT[('k', b)] = np.ascontiguousarray(K[b].T).astype(BF16)
        xT[('v', b)] = np.ascontiguousarray(V[b].T).astype(BF16)
    ident_np = np.eye(P, dtype=BF16)

    in_maps = []
    for c in range(NCORES):
        b, g = c // GROUPS, c % GROUPS
        sl = slice(g * GD, (g + 1) * GD)
        in_maps.append({
            "xqT": xT[('q', b)],
            "xkT": xT[('k', b)],
            "xvT": xT[('v', b)],
            "wq": np.ascontiguousarray(Wq[:, sl]).astype(BF16),
            "wk": np.ascontiguousarray(Wk[:, sl]).astype(BF16),
            "wv": np.ascontiguousarray(Wv[:, sl]).astype(BF16),
            "wo": np.ascontiguousarray(Wo[sl, :]).astype(BF16),
            "bq": np.ascontiguousarray(bq[sl].reshape(GD, 1)),
            "bk": np.ascontiguousarray(bk[sl].reshape(GD, 1)),
            "ident": ident_np,
        })

    if "nc" not in _cached:
        _cached["nc"] = _build_bass()
    nc = _cached["nc"]

    try:
        res = run_bass_kernel_spmd(nc, in_maps, core_ids=list(range(NCORES)))
    except ModuleNotFoundError:
        # BASS_TRACE set but the axon ntff hook isn't shipped in this
        # container - retry untraced
        os.environ["BASS_NEVER_TRACE"] = "1"
        res = run_bass_kernel_spmd(nc, in_maps, core_ids=list(range(NCORES)))
    if res.exec_time_ns is not None:
        print(f"HW exec time: {res.exec_time_ns} ns")

    bo_eff = (bv @ Wo + bo).astype(f32)
    out = np.zeros((B, S, D), f32)
    for c in range(NCORES):
        b = c // GROUPS
        out[b] += np.asarray(res.results[c]["out"], f32)
    out += bo_eff
    return out
